# revision 70
# baseline (speedup 1.0000x reference)
"""Bidirectional 2-layer LSTM (with replicated hf1-input bug) + per-step linear,
as a Trainium2 Bass/Tile kernel, data-parallel over batch across 8 NeuronCores.

v4: fp16 datapath, two phase-shifted half-batch pipelines per core with the
elementwise chains split across DVE (half A) and GpSimd (half B), and a
transpose-free output path.

Layout strategy (per core, B_loc=256 batch split into halves A/B of 128):
  - packed state tile pkq [128 rows, 4 slots, 2 halves, 128 batch] fp16:
      rows 0:32 hf0, 32:64 hf1, 64:96 hb0; row 96 = x[t], 97 = xb[t],
      98 = ones (bias enters via the ones row).
  - per half-wave: 4 fp16 matmuls (one per gate i,f,g,o), K=128 x M=128 x
    N=128, into a per-half PSUM bank; the g-gate weights are pre-scaled by 2
    so one merged Sigmoid yields sigma(2g), and tanh(g) = 2*sigma(2g)-1.
  - Scalar engine: one Sigmoid over [128, 512] + one Tanh over the fp16 cell
    state [128, 128] per half-wave (the bottleneck engine).
  - Elementwise cell update (wt = 2*sig_2g - 1 = tanh(g); v = sig_f*c;
    u3 = wt*sig_i; c = u3 + v; h2 = sig_o * tanh(c)): half A on DVE, half B
    on GpSimd, so the two pipelines never head-of-line block each other.
  - Output projection, transpose-free: per wave t two N=1 matmuls
    (lhsT = pkq slot / hb1 slot as stationary [K=128, M=128 batch],
    rhs = wout column) accumulate out[t]'s 128-batch column directly in
    batch-major PSUM ring tiles obr[h][(t//4)%2][:, t%4] (one full bank per
    tile, wo2-close emitted before wo1-open so PSUM accumulation groups
    never interleave).  DVE copies each completed 4-column group to an SBUF
    staging tile; every 16 waves one DMA writes an aligned [128, 2, 8]
    block to out[h, b, t//8, t%8] in DRAM (dynamic register index on the
    DRAM dim only; both DMA APs kept <= 3 dims — HW DGE cannot execute 4D
    APs even though they compile).
  - The hardware loop covers 32 waves per iteration (amortizing the For_i
    all-engine barrier) while keeping <= 6 dynamic base+const DMA index
    expressions per queue (more breaks the AP lowering), via a parity-split
    xg layout indexed in 2-group units.
"""

import sys

sys.path.insert(0, "/opt/trn_rl_repo")

import numpy as np
import concourse.bass as bass
import concourse.tile as tile
import concourse.mybir as mybir
import bass_rust
from concourse.bass_utils import run_bass_kernel_spmd

S, B, H = 1024, 2048, 32
NCORES = 8
BL = B // NCORES  # 256 per-core batch
HB = BL // 2      # 128 half-batch

F32 = mybir.dt.float32
F16 = mybir.dt.float16
AF = mybir.ActivationFunctionType
OP = mybir.AluOpType

# cell order along M-columns / state partitions: [f0, f1, b0, b1]
CELL_COL = {"f0": 0, "f1": 32, "b0": 64, "b1": 96}
ROW_HF0, ROW_HF1, ROW_HB0 = 0, 32, 64
ROW_X, ROW_XB, ROW_ONES = 96, 97, 98


def _split_excess_waits(nc, max_waits=1):
    """walrus codegen in this toolchain supports only one sync-wait per
    instruction; split extras onto inserted wait-only drains."""
    n = 0
    for f in nc.m.functions:
        for bb in f.blocks:
            newl = []
            dirty = False
            for ins in bb.instructions:
                si = ins.sync_info
                waits = list(si.on_wait) if si is not None else []
                if len(waits) > max_waits:
                    dirty = True
                    k = len(waits) - max_waits
                    i = 0
                    while i < k:
                        chunk = waits[i : min(i + max_waits, k)]
                        d = mybir.InstDrain(name=f"zwsplit-{n}", is_reset_sema=False)
                        n += 1
                        d.engine = ins.engine
                        d.sync_info = bass_rust.SyncInfo(on_wait=chunk, on_update=[])
                        newl.append(d)
                        i += max_waits
                    si.on_wait = waits[k:]
                    ins.sync_info = si
                newl.append(ins)
            if dirty:
                bb.instructions = newl
    return n


def _gate_block(Wmat, gi):
    """rows of a torch 4H-row weight/bias for gate gi (torch order i,f,g,o)."""
    return Wmat[gi * H : (gi + 1) * H]


def build_weights(Wih_f0, Whh_f0, b_f0, Wih_f1, Whh_f1, b_f1,
                  Wih_b0, Whh_b0, b_b0, Wih_b1, Whh_b1, b_b1, Wlin, blin):
    """Pack per-gate stationary matrices Wg -> [K=128, gate, M=128] plus the
    two output-projection columns (all fp16)."""
    Wg = np.zeros((4, 128, 128), np.float32)
    for gi in range(4):
        sc = 2.0 if gi == 2 else 1.0  # tanh-gate pre-scale
        c = CELL_COL["f0"]  # inp = x, h = hf0
        Wg[gi, ROW_X, c : c + H] = _gate_block(Wih_f0, gi)[:, 0] * sc
        Wg[gi, ROW_ONES, c : c + H] = _gate_block(b_f0, gi) * sc
        Wg[gi, ROW_HF0 : ROW_HF0 + H, c : c + H] = _gate_block(Whh_f0, gi).T * sc
        c = CELL_COL["f1"]  # inp = hf0, h = hf1
        Wg[gi, ROW_ONES, c : c + H] = _gate_block(b_f1, gi) * sc
        Wg[gi, ROW_HF0 : ROW_HF0 + H, c : c + H] = _gate_block(Wih_f1, gi).T * sc
        Wg[gi, ROW_HF1 : ROW_HF1 + H, c : c + H] = _gate_block(Whh_f1, gi).T * sc
        c = CELL_COL["b0"]  # inp = xb, h = hb0
        Wg[gi, ROW_XB, c : c + H] = _gate_block(Wih_b0, gi)[:, 0] * sc
        Wg[gi, ROW_ONES, c : c + H] = _gate_block(b_b0, gi) * sc
        Wg[gi, ROW_HB0 : ROW_HB0 + H, c : c + H] = _gate_block(Whh_b0, gi).T * sc
        c = CELL_COL["b1"]  # inp = hb0, h-arg = hf1 (replicated bug)
        Wg[gi, ROW_ONES, c : c + H] = _gate_block(b_b1, gi) * sc
        Wg[gi, ROW_HB0 : ROW_HB0 + H, c : c + H] = _gate_block(Wih_b1, gi).T * sc
        Wg[gi, ROW_HF1 : ROW_HF1 + H, c : c + H] = _gate_block(Whh_b1, gi).T * sc

    wout1 = np.zeros((128, 1), np.float32)
    wout1[ROW_ONES, 0] = blin[0]
    wout1[ROW_HF1 : ROW_HF1 + H, 0] = Wlin[0, 0:H]
    wout2 = np.zeros((128, 1), np.float32)
    wout2[96:128, 0] = Wlin[0, H : 2 * H]
    return (np.ascontiguousarray(Wg.transpose(1, 0, 2)).astype(np.float16),
            wout1.astype(np.float16), wout2.astype(np.float16))


def build_xg(x_shard, s):
    """Group-packed x rows: xg[G, h, r, j, :] is packed-partition row 96+r
    (0 = x, 1 = xb, 2 = ones) of wave w = 4G + j for half h."""
    bl = x_shard.shape[1]
    hb = bl // 2
    ngroup = s // 4 + 2
    xg = np.zeros((ngroup, 2, 3, 4, hb), np.float16)
    xg[:, :, 2] = 1.0
    x16 = x_shard.astype(np.float16)
    xh = x16.reshape(s, 2, hb)
    # x rows: wave w < s
    xg[0 : s // 4, :, 0, :, :] = xh.reshape(s // 4, 4, 2, hb).transpose(0, 2, 1, 3)
    # xb rows: wave w in 1..s+1 reads x[(s + 1 - w) % s]
    w = np.arange(1, s + 2)
    xb = xh[(s + 1 - w) % s]  # [s+1, 2, hb]
    xbp = np.zeros((ngroup * 4, 2, hb), np.float16)
    xbp[1 : s + 2] = xb
    xg[:, :, 1] = xbp.reshape(ngroup, 4, 2, hb).transpose(0, 2, 1, 3)
    # parity-split group dim: [q, par, half, row, slot, batch], g = 2q + par,
    # with 2 leading pad q-entries (see xd declaration).
    xq = np.ascontiguousarray(xg).reshape(ngroup // 2, 2, 2, 3, 4, hb)
    return np.concatenate([np.zeros_like(xq[:2]), xq], axis=0)


def build_nc(s=S, dbg=False, split_waits=True):
    assert s % 128 == 0 and (s // 4 - 8) % 8 == 0
    nc = bass.Bass("TRN2", target_bir_lowering=False, debug=False,
                   num_devices=NCORES)

    ngroup = s // 4 + 2
    # group g = 2q + par; the parity split keeps every dynamic DMA index in
    # 2-group units so the per-queue base+const expression variety stays low.
    # 2 leading pad entries keep the xg index-expression VALUES disjoint
    # from the out-store ones (same value + same queue breaks AP lowering).
    xd = nc.declare_dram_parameter("xg", [ngroup // 2 + 2, 2, 2, 3, 4, HB],
                                   F16, isOutput=False)
    wgd = nc.declare_dram_parameter("Wg", [128, 4, 128], F16, isOutput=False)
    wo1d = nc.declare_dram_parameter("wout1", [128, 1], F16, isOutput=False)
    wo2d = nc.declare_dram_parameter("wout2", [128, 1], F16, isOutput=False)
    # out[h, b, q8, r] = output for batch row h*HB+b at t = 8*q8 + r
    # (host reshapes to [BL, s] directly).  The last dim is 8 wide so every
    # out-store DMA has 2-3 dim APs on BOTH sides — the hardware DGE cannot
    # execute 4D APs even though they compile.
    outd = nc.declare_dram_parameter("out", [2, HB, s // 8, 8], F32,
                                     isOutput=True)

    with tile.TileContext(nc) as tc:
        with (
            tc.tile_pool(name="const", bufs=1) as cpool,
            tc.tile_pool(name="state", bufs=1) as spool,
            tc.tile_pool(name="psum", bufs=1, space="PSUM") as ppool,
        ):
            wg_t = cpool.tile([128, 4, 128], F16)
            wo1_t = cpool.tile([128, 1], F16)
            wo2_t = cpool.tile([128, 1], F16)
            nc.sync.dma_start(wg_t[:], wgd[:])
            nc.sync.dma_start(wo1_t[:], wo1d[:])
            nc.sync.dma_start(wo2_t[:], wo2d[:])

            # Every piece of cross-engine state is a separate tile per half:
            # the tile dependency tracker is coarse per-tile, and a shared
            # tile serializes the two half-batch pipelines against each
            # other.  pkq and hb1 are 8-deep rings (slot = wave mod 8).
            pkq_g = [[spool.tile([128, 4, HB], F16, name=f"pkq{hh}_{gg}")
                      for gg in range(2)] for hh in range(2)]
            c_h = [spool.tile([128, 2, HB], F16, name=f"c{hh}")
                   for hh in range(2)]
            sig_p = [[spool.tile([128, 4 * HB], F16, name=f"sig{hh}_{pp}")
                      for pp in range(4)] for hh in range(2)]
            tct_p = [[spool.tile([128, HB], F16, name=f"tct{hh}_{pp}")
                      for pp in range(4)] for hh in range(2)]
            u3_h = [spool.tile([128, HB], F16, name=f"u3{hh}")
                    for hh in range(2)]
            v_h = [spool.tile([128, HB], F16, name=f"v{hh}")
                   for hh in range(2)]
            wt_h = [spool.tile([128, HB], F16, name=f"wt{hh}")
                    for hh in range(2)]
            hb1_g = [[spool.tile([128, 4, HB], F16, name=f"hb1{hh}_{gg}")
                      for gg in range(2)] for hh in range(2)]
            # staging for 16 out columns: [128, 2 q8-blocks, 8 t-cols]
            stg = [spool.tile([128, 2, 8], F32, name=f"stg{hh}")
                   for hh in range(2)]

            gps_h = [ppool.tile([128, 4 * HB], F32, name=f"gps{hh}")
                     for hh in range(2)]
            # out-projection ring: obr[h][i][:, k] accumulates out column
            # t with (t//4)%2 == i, t%4 == k for half h.  Tiles are padded
            # to a full PSUM bank so accumulation-group tracking never
            # crosses tiles; only cols 0:4 are used.
            obr = [[ppool.tile([128, 512], F32, name=f"obr{hh}_{ii}")
                    for ii in range(2)] for hh in range(2)]

            # ---- init ----
            for hh in range(2):
                for gg in range(2):
                    nc.vector.memset(pkq_g[hh][gg][:], 0.0)
                    nc.vector.memset(hb1_g[hh][gg][:], 0.0)
                nc.vector.memset(c_h[hh][:], 0.0)

            def phase1(wm, h):
                """wm = wave index mod 8."""
                pb = wm % 2
                ps = wm % 4
                p = pkq_g[h][wm // 4][:, wm % 4, :]
                for g in range(4):
                    nc.tensor.matmul(
                        gps_h[h][:, g * HB : (g + 1) * HB],
                        wg_t[:, g, :], p, start=True, stop=True,
                    )
                nc.scalar.activation(sig_p[h][ps][:], gps_h[h][:],
                                     AF.Sigmoid)
                i_s = sig_p[h][ps][:, 0:HB]
                f_s = sig_p[h][ps][:, HB : 2 * HB]
                g_s = sig_p[h][ps][:, 2 * HB : 3 * HB]
                # v = sig_f * c ; u3 = (2*sig_2g - 1) * sig_i = tanh(g)*sig_i
                # Half A's elementwise chain runs on DVE, half B's on Pool,
                # so the two pipelines never head-of-line block each other.
                eng = nc.vector if h == 0 else nc.gpsimd
                eng.tensor_scalar(
                    wt_h[h][:], g_s, 2.0, -1.0, OP.mult, OP.add)
                eng.tensor_tensor(
                    v_h[h][:], f_s, c_h[h][:, 1 - pb, :], OP.mult)
                eng.tensor_tensor(
                    u3_h[h][:], wt_h[h][:], i_s, OP.mult)
                eng.tensor_tensor(
                    c_h[h][:, pb, :], u3_h[h][:], v_h[h][:], OP.add)

            def phase2(wm, h):
                pb = wm % 2
                ps = wm % 4
                nc.scalar.activation(tct_p[h][ps][:], c_h[h][:, pb, :],
                                     AF.Tanh)
                w2 = (wm + 1) % 8
                eng = nc.vector if h == 0 else nc.gpsimd
                eng.tensor_tensor(
                    pkq_g[h][w2 // 4][0:96, w2 % 4, :],
                    sig_p[h][ps][0:96, 3 * HB : 4 * HB],
                    tct_p[h][ps][0:96, :], OP.mult,
                )
                eng.tensor_tensor(
                    hb1_g[h][wm // 4][96:128, wm % 4, :],
                    sig_p[h][ps][96:128, 3 * HB : 4 * HB],
                    tct_p[h][ps][96:128, :], OP.mult,
                )

            def xload(grp, s4):
                """load x rows of group grp (waves 4g..4g+3) into slots
                s4..s4+4, one DMA per half (SP for A, ACT queue for B).
                grp: int, or (q_dsexpr, parity) with grp = 2q + parity."""
                for h, eng in ((0, nc.sync), (1, nc.scalar)):
                    if isinstance(grp, int):
                        q0 = grp // 2 + 2
                        src = xd[q0 : q0 + 1, grp % 2, h, :, :, :]
                    else:
                        q, par = grp
                        src = xd[q, par, h, :, :, :]
                    eng.dma_start(pkq_g[h][s4 // 4][96:99, 0:4, :], src)

            def full_wave(wm, wo1i=None, wo2i=None, xl=None, cp=None,
                          dma_q=None):
                """wo1i/wo2i: (ring_tile_idx, col) for out rows t=w-2 / t=w-3.
                cp: (ring_tile_idx, t-col-base-is-implicit) staging copy.
                dma_q: DRAM q index (int or register expr) for an 8-col block.
                """
                for h in range(2):
                    phase1(wm, h)
                # wo2 (closing column t=w-3) MUST precede wo1 (opening
                # column t=w-2): PSUM accumulation groups on a tile must be
                # strictly sequential, never interleaved.
                if wo2i is not None:
                    ii, kk = wo2i
                    s2 = (wm - 1) % 8
                    for h in range(2):
                        nc.tensor.matmul(
                            obr[h][ii][:, kk : kk + 1],
                            hb1_g[h][s2 // 4][:, s2 % 4, :], wo2_t[:],
                            start=False, stop=True, skip_group_check=True)
                if wo1i is not None:
                    ii, kk = wo1i
                    for h in range(2):
                        nc.tensor.matmul(
                            obr[h][ii][:, kk : kk + 1],
                            pkq_g[h][wm // 4][:, wm % 4, :], wo1_t[:],
                            start=True, stop=False, skip_group_check=True)
                for h in range(2):
                    phase2(wm, h)
                if xl is not None:
                    xload(*xl)
                # evac traffic goes AFTER phase2 so it never delays the
                # critical h2 ops in the DVE/Pool queues.
                if cp is not None:
                    ti, sl = cp
                    for h in range(2):
                        nc.vector.tensor_copy(
                            stg[h][:, sl // 2, 4 * (sl % 2) : 4 * (sl % 2) + 4],
                            obr[h][ti][:, 0:4])
                if dma_q is not None:
                    for h, eng in ((0, nc.sync), (1, nc.scalar)):
                        eng.dma_start(
                            outd[h, :, dma_q : dma_q + 2, :]
                            if isinstance(dma_q, int)
                            else outd[h, :, bass.ds(dma_q, 2), :],
                            stg[h][:])

            def wave_kwargs_static(w):
                """out/copy/dma/xload schedule for a statically-indexed wave."""
                kw = {}
                t1 = w - 2
                if 0 <= t1 < s:
                    kw["wo1i"] = ((t1 // 4) % 2, t1 % 4)
                t2 = w - 3
                if 0 <= t2 < s:
                    kw["wo2i"] = ((t2 // 4) % 2, t2 % 4)
                if w % 4 == 3 and (w + 5) // 4 < ngroup:
                    kw["xl"] = ((w + 5) // 4, 0 if w % 8 == 3 else 4)
                if w % 8 == 7 and w >= 7 and w - 7 < s:
                    # ring tile 0 holds t = w-7 .. w-4
                    kw["cp"] = (0, ((w - 7) // 4) % 4)
                if w % 8 == 3 and w >= 11 and w - 7 < s:
                    # ring tile 1 holds t = w-7 .. w-4
                    kw["cp"] = (1, ((w - 7) // 4) % 4)
                    if w % 16 == 3 and w >= 19:
                        kw["dma_q"] = (w - 19) // 8
                return kw

            # ---- prologue: waves 0,1 (skew priming, no out) ----
            xload(0, 0)
            xload(1, 4)
            full_wave(0)
            # zero junk written into hf1/hb0 rows of pkq slot 1 and cf1/cb0
            for hh in range(2):
                nc.vector.memset(pkq_g[hh][0][32:64, 1, :], 0.0)
                nc.vector.memset(pkq_g[hh][0][64:96, 1, :], 0.0)
                nc.vector.memset(c_h[hh][32:64, 0, :], 0.0)
                nc.vector.memset(c_h[hh][64:96, 0, :], 0.0)
            full_wave(1)
            for hh in range(2):
                nc.vector.memset(c_h[hh][96:128, 1, :], 0.0)

            # ---- static waves 2..33 (peel: groups 0..7 compute) ----
            for w in range(2, 34):
                full_wave(w % 8, **wave_kwargs_static(w))

            def dyn_body(iqb, goff):
                """2 groups = 8 waves: G = 2*iqb + goff, w = 4G+2 .. 4G+9.
                iqb is the loop register in 2-group units; every dynamic
                index is a single `iqb + const`."""
                for idx in range(8):
                    w_off = 2 + idx
                    wm = w_off % 8
                    kw = {}
                    kw["wo1i"] = ((idx // 4) % 2, idx % 4)
                    kw["wo2i"] = (((idx - 1) // 4) % 2, (idx - 1) % 4)
                    if wm == 3:
                        kw["xl"] = ((bass.ds(iqb + goff // 2 + 3, 1), 0), 0)
                        kw["cp"] = (1, (goff - 1) % 4)
                        if goff % 4 == 0:
                            kw["dma_q"] = iqb + goff // 2 - 2
                    elif wm == 7:
                        kw["xl"] = ((bass.ds(iqb + goff // 2 + 3, 1), 1), 4)
                        kw["cp"] = (0, goff % 4)
                    full_wave(wm, **kw)

            # ---- main loop: groups 8 .. s//4-2, 8 groups per iteration.
            # The loop register iq counts 2-group units; parity-split xg and
            # 16-column out blocks keep the per-queue dynamic-DMA expression
            # variety at 6 (the AP lowering chokes on too many base+const
            # variants per queue, and walrus cannot codegen gpsimd SWDGE
            # DMAs at all), while 32-wave bodies amortize the per-iteration
            # all-engine barrier.
            with tc.For_i(4, s // 8, 4) as iq:
                for g4 in range(4):
                    dyn_body(iq, 2 * g4)

            # ---- epilogue ----
            # last out row t = s-1: wo2 against hb1 slot (s+1)%8 = 1
            for h in range(2):
                nc.tensor.matmul(
                    obr[h][1][:, 3:4], hb1_g[h][0][:, 1, :], wo2_t[:],
                    start=False, stop=True, skip_group_check=True)
            # evacuate the final 16 columns (t = s-16 .. s-1): staging slots
            # 0..2 were filled in-loop; copy ring tile 1 into slot 3, then
            # one DMA for q8 pair s//8-2.
            for h in range(2):
                nc.vector.tensor_copy(stg[h][:, 1, 4:8], obr[h][1][:, 0:4])
            for h, eng in ((0, nc.sync), (1, nc.scalar)):
                eng.dma_start(
                    outd[h, :, s // 8 - 2 : s // 8, :], stg[h][:])

    if split_waits:
        _split_excess_waits(nc)
    return nc


_NC_CACHE = {}


def _get_nc(s=S, dbg=False):
    key = (s, dbg)
    if key not in _NC_CACHE:
        _NC_CACHE[key] = build_nc(s, dbg)
    return _NC_CACHE[key]


def core_inputs(x, weights, s=S, core=0):
    """Input map for one core. x: [s, B] fp32 (already squeezed)."""
    Wg, wout1, wout2 = build_weights(**weights)
    xs = np.ascontiguousarray(x[:, core * BL : (core + 1) * BL])
    return {"xg": build_xg(xs, s),
            "Wg": Wg, "wout1": wout1, "wout2": wout2}


def run(x, weights, s=S, dbg=False, trace=False):
    """x: [s, B] fp32 (already squeezed); weights: dict of reference arrays."""
    Wg, wout1, wout2 = build_weights(**weights)
    nc = _get_nc(s, dbg)
    in_maps = []
    for c in range(NCORES):
        xs = np.ascontiguousarray(x[:, c * BL : (c + 1) * BL])
        in_maps.append(
            {"xg": build_xg(xs, s),
             "Wg": Wg, "wout1": wout1, "wout2": wout2}
        )
    res = run_bass_kernel_spmd(nc, in_maps, list(range(NCORES)), trace=trace)
    out = np.concatenate(
        [res.results[c]["out"].reshape(BL, s) for c in range(NCORES)], axis=0)
    return out, res


def kernel(x, Wih_f0, Whh_f0, b_f0, Wih_f1, Whh_f1, b_f1,
           Wih_b0, Whh_b0, b_b0, Wih_b1, Whh_b1, b_b1, Wlin, blin, future):
    assert int(future) == 0, "kernel hardcodes future=0"
    x = np.asarray(x, np.float32)
    s, b, _ = x.shape
    assert (s, b) == (S, B)
    weights = dict(
        Wih_f0=np.asarray(Wih_f0, np.float32), Whh_f0=np.asarray(Whh_f0, np.float32),
        b_f0=np.asarray(b_f0, np.float32),
        Wih_f1=np.asarray(Wih_f1, np.float32), Whh_f1=np.asarray(Whh_f1, np.float32),
        b_f1=np.asarray(b_f1, np.float32),
        Wih_b0=np.asarray(Wih_b0, np.float32), Whh_b0=np.asarray(Whh_b0, np.float32),
        b_b0=np.asarray(b_b0, np.float32),
        Wih_b1=np.asarray(Wih_b1, np.float32), Whh_b1=np.asarray(Whh_b1, np.float32),
        b_b1=np.asarray(b_b1, np.float32),
        Wlin=np.asarray(Wlin, np.float32), blin=np.asarray(blin, np.float32),
    )
    out, _ = run(x[:, :, 0], weights, s=S)
    return out


# revision 84
# speedup vs baseline: 1.0019x; 1.0019x over previous
"""Bidirectional 2-layer LSTM (with replicated hf1-input bug) + per-step linear,
as a Trainium2 Bass/Tile kernel, data-parallel over batch across 8 NeuronCores.

v4: fp16 datapath, two phase-shifted half-batch pipelines per core with the
elementwise chains split across DVE (half A) and GpSimd (half B), and a
transpose-free output path.

Layout strategy (per core, B_loc=256 batch split into halves A/B of 128):
  - packed state tile pkq [128 rows, 4 slots, 2 halves, 128 batch] fp16:
      rows 0:32 hf0, 32:64 hf1, 64:96 hb0; row 96 = x[t], 97 = xb[t],
      98 = ones (bias enters via the ones row).
  - per half-wave: 4 fp16 matmuls (one per gate i,f,g,o), K=128 x M=128 x
    N=128, into a per-half PSUM bank; the g-gate weights are pre-scaled by 2
    so one merged Sigmoid yields sigma(2g), and tanh(g) = 2*sigma(2g)-1.
  - Scalar engine: one Sigmoid over [128, 512] + one Tanh over the fp16 cell
    state [128, 128] per half-wave (the bottleneck engine).
  - Elementwise cell update (wt = 2*sig_2g - 1 = tanh(g); v = sig_f*c;
    u3 = wt*sig_i; c = u3 + v; h2 = sig_o * tanh(c)): half A on DVE, half B
    on GpSimd, so the two pipelines never head-of-line block each other.
  - Output projection, transpose-free: per wave t two N=1 matmuls
    (lhsT = pkq slot / hb1 slot as stationary [K=128, M=128 batch],
    rhs = wout column) accumulate out[t]'s 128-batch column directly in
    batch-major PSUM ring tiles obr[h][(t//4)%2][:, t%4] (one full bank per
    tile, wo2-close emitted before wo1-open so PSUM accumulation groups
    never interleave).  DVE copies each completed 4-column group to an SBUF
    staging tile; every 16 waves one DMA writes an aligned [128, 2, 8]
    block to out[h, b, t//8, t%8] in DRAM (dynamic register index on the
    DRAM dim only; both DMA APs kept <= 3 dims — HW DGE cannot execute 4D
    APs even though they compile).
  - The hardware loop covers 32 waves per iteration (amortizing the For_i
    all-engine barrier) while keeping <= 6 dynamic base+const DMA index
    expressions per queue (more breaks the AP lowering), via a parity-split
    xg layout indexed in 2-group units.
"""

import sys

sys.path.insert(0, "/opt/trn_rl_repo")

import numpy as np
import concourse.bass as bass
import concourse.tile as tile
import concourse.mybir as mybir
import bass_rust
from concourse.bass_utils import run_bass_kernel_spmd

S, B, H = 1024, 2048, 32
NCORES = 8
BL = B // NCORES  # 256 per-core batch
HB = BL // 2      # 128 half-batch

F32 = mybir.dt.float32
F16 = mybir.dt.float16
AF = mybir.ActivationFunctionType
OP = mybir.AluOpType

# cell order along M-columns / state partitions: [f0, f1, b0, b1]
CELL_COL = {"f0": 0, "f1": 32, "b0": 64, "b1": 96}
ROW_HF0, ROW_HF1, ROW_HB0 = 0, 32, 64
ROW_X, ROW_XB, ROW_ONES = 96, 97, 98


def _split_excess_waits(nc, max_waits=1):
    """walrus codegen in this toolchain supports only one sync-wait per
    instruction; split extras onto inserted wait-only drains."""
    n = 0
    for f in nc.m.functions:
        for bb in f.blocks:
            newl = []
            dirty = False
            for ins in bb.instructions:
                si = ins.sync_info
                waits = list(si.on_wait) if si is not None else []
                if len(waits) > max_waits:
                    dirty = True
                    k = len(waits) - max_waits
                    i = 0
                    while i < k:
                        chunk = waits[i : min(i + max_waits, k)]
                        d = mybir.InstDrain(name=f"zwsplit-{n}", is_reset_sema=False)
                        n += 1
                        d.engine = ins.engine
                        d.sync_info = bass_rust.SyncInfo(on_wait=chunk, on_update=[])
                        newl.append(d)
                        i += max_waits
                    si.on_wait = waits[k:]
                    ins.sync_info = si
                newl.append(ins)
            if dirty:
                bb.instructions = newl
    return n


def _gate_block(Wmat, gi):
    """rows of a torch 4H-row weight/bias for gate gi (torch order i,f,g,o)."""
    return Wmat[gi * H : (gi + 1) * H]


def build_weights(Wih_f0, Whh_f0, b_f0, Wih_f1, Whh_f1, b_f1,
                  Wih_b0, Whh_b0, b_b0, Wih_b1, Whh_b1, b_b1, Wlin, blin):
    """Pack per-gate stationary matrices Wg -> [K=128, gate, M=128] plus the
    two output-projection columns (all fp16)."""
    Wg = np.zeros((4, 128, 128), np.float32)
    for gi in range(4):
        sc = 2.0 if gi == 2 else 1.0  # tanh-gate pre-scale
        c = CELL_COL["f0"]  # inp = x, h = hf0
        Wg[gi, ROW_X, c : c + H] = _gate_block(Wih_f0, gi)[:, 0] * sc
        Wg[gi, ROW_ONES, c : c + H] = _gate_block(b_f0, gi) * sc
        Wg[gi, ROW_HF0 : ROW_HF0 + H, c : c + H] = _gate_block(Whh_f0, gi).T * sc
        c = CELL_COL["f1"]  # inp = hf0, h = hf1
        Wg[gi, ROW_ONES, c : c + H] = _gate_block(b_f1, gi) * sc
        Wg[gi, ROW_HF0 : ROW_HF0 + H, c : c + H] = _gate_block(Wih_f1, gi).T * sc
        Wg[gi, ROW_HF1 : ROW_HF1 + H, c : c + H] = _gate_block(Whh_f1, gi).T * sc
        c = CELL_COL["b0"]  # inp = xb, h = hb0
        Wg[gi, ROW_XB, c : c + H] = _gate_block(Wih_b0, gi)[:, 0] * sc
        Wg[gi, ROW_ONES, c : c + H] = _gate_block(b_b0, gi) * sc
        Wg[gi, ROW_HB0 : ROW_HB0 + H, c : c + H] = _gate_block(Whh_b0, gi).T * sc
        c = CELL_COL["b1"]  # inp = hb0, h-arg = hf1 (replicated bug)
        Wg[gi, ROW_ONES, c : c + H] = _gate_block(b_b1, gi) * sc
        Wg[gi, ROW_HB0 : ROW_HB0 + H, c : c + H] = _gate_block(Wih_b1, gi).T * sc
        Wg[gi, ROW_HF1 : ROW_HF1 + H, c : c + H] = _gate_block(Whh_b1, gi).T * sc

    wout1 = np.zeros((128, 1), np.float32)
    wout1[ROW_ONES, 0] = blin[0]
    wout1[ROW_HF1 : ROW_HF1 + H, 0] = Wlin[0, 0:H]
    wout2 = np.zeros((128, 1), np.float32)
    wout2[96:128, 0] = Wlin[0, H : 2 * H]
    return (np.ascontiguousarray(Wg.transpose(1, 0, 2)).astype(np.float16),
            wout1.astype(np.float16), wout2.astype(np.float16))


def build_xg(x_shard, s):
    """Group-packed x rows: xg[G, h, r, j, :] is packed-partition row 96+r
    (0 = x, 1 = xb, 2 = ones) of wave w = 4G + j for half h."""
    bl = x_shard.shape[1]
    hb = bl // 2
    ngroup = s // 4 + 2
    xg = np.zeros((ngroup, 2, 3, 4, hb), np.float16)
    xg[:, :, 2] = 1.0
    x16 = x_shard.astype(np.float16)
    xh = x16.reshape(s, 2, hb)
    # x rows: wave w < s
    xg[0 : s // 4, :, 0, :, :] = xh.reshape(s // 4, 4, 2, hb).transpose(0, 2, 1, 3)
    # xb rows: wave w in 1..s+1 reads x[(s + 1 - w) % s]
    w = np.arange(1, s + 2)
    xb = xh[(s + 1 - w) % s]  # [s+1, 2, hb]
    xbp = np.zeros((ngroup * 4, 2, hb), np.float16)
    xbp[1 : s + 2] = xb
    xg[:, :, 1] = xbp.reshape(ngroup, 4, 2, hb).transpose(0, 2, 1, 3)
    # parity-split group dim: [q, par, half, row, slot, batch], g = 2q + par,
    # with 2 leading pad q-entries (see xd declaration).
    xq = np.ascontiguousarray(xg).reshape(ngroup // 2, 2, 2, 3, 4, hb)
    return np.concatenate([np.zeros_like(xq[:2]), xq], axis=0)


def build_nc(s=S, dbg=False, split_waits=True):
    assert s % 128 == 0 and (s // 4 - 8) % 8 == 0
    nc = bass.Bass("TRN2", target_bir_lowering=False, debug=False,
                   num_devices=NCORES)

    ngroup = s // 4 + 2
    # group g = 2q + par; the parity split keeps every dynamic DMA index in
    # 2-group units so the per-queue base+const expression variety stays low.
    # 2 leading pad entries keep the xg index-expression VALUES disjoint
    # from the out-store ones (same value + same queue breaks AP lowering).
    xd = nc.declare_dram_parameter("xg", [ngroup // 2 + 2, 2, 2, 3, 4, HB],
                                   F16, isOutput=False)
    wgd = nc.declare_dram_parameter("Wg", [128, 4, 128], F16, isOutput=False)
    wo1d = nc.declare_dram_parameter("wout1", [128, 1], F16, isOutput=False)
    wo2d = nc.declare_dram_parameter("wout2", [128, 1], F16, isOutput=False)
    # out[h, b, q8, r] = output for batch row h*HB+b at t = 8*q8 + r
    # (host reshapes to [BL, s] directly).  The last dim is 8 wide so every
    # out-store DMA has 2-3 dim APs on BOTH sides — the hardware DGE cannot
    # execute 4D APs even though they compile.
    outd = nc.declare_dram_parameter("out", [2, HB, s // 8, 8], F32,
                                     isOutput=True)

    with tile.TileContext(nc) as tc:
        with (
            tc.tile_pool(name="const", bufs=1) as cpool,
            tc.tile_pool(name="state", bufs=1) as spool,
            tc.tile_pool(name="psum", bufs=1, space="PSUM") as ppool,
        ):
            wg_t = cpool.tile([128, 4, 128], F16)
            wo1_t = cpool.tile([128, 1], F16)
            wo2_t = cpool.tile([128, 1], F16)
            nc.sync.dma_start(wg_t[:], wgd[:])
            nc.sync.dma_start(wo1_t[:], wo1d[:])
            nc.sync.dma_start(wo2_t[:], wo2d[:])

            # Every piece of cross-engine state is a separate tile per half:
            # the tile dependency tracker is coarse per-tile, and a shared
            # tile serializes the two half-batch pipelines against each
            # other.  pkq and hb1 are 8-deep rings (slot = wave mod 8).
            pkq_g = [[spool.tile([128, 4, HB], F16, name=f"pkq{hh}_{gg}")
                      for gg in range(2)] for hh in range(2)]
            c_h = [spool.tile([128, 2, HB], F16, name=f"c{hh}")
                   for hh in range(2)]
            sig_p = [[spool.tile([128, 4 * HB], F16, name=f"sig{hh}_{pp}")
                      for pp in range(4)] for hh in range(2)]
            tct_p = [[spool.tile([128, HB], F16, name=f"tct{hh}_{pp}")
                      for pp in range(4)] for hh in range(2)]
            u3_h = [spool.tile([128, HB], F16, name=f"u3{hh}")
                    for hh in range(2)]
            v_h = [spool.tile([128, HB], F16, name=f"v{hh}")
                   for hh in range(2)]
            wt_h = [spool.tile([128, HB], F16, name=f"wt{hh}")
                    for hh in range(2)]
            hb1_g = [[spool.tile([128, 4, HB], F16, name=f"hb1{hh}_{gg}")
                      for gg in range(2)] for hh in range(2)]
            # staging for 16 out columns: [128, 2 q8-blocks, 8 t-cols]
            stg = [spool.tile([128, 2, 8], F32, name=f"stg{hh}")
                   for hh in range(2)]

            gps_h = [ppool.tile([128, 4 * HB], F32, name=f"gps{hh}")
                     for hh in range(2)]
            # out-projection ring: obr[h][i][:, k] accumulates out column
            # t with (t//4)%2 == i, t%4 == k for half h.  Tiles are padded
            # to a full PSUM bank so accumulation-group tracking never
            # crosses tiles; only cols 0:4 are used.
            obr = [[ppool.tile([128, 512], F32, name=f"obr{hh}_{ii}")
                    for ii in range(2)] for hh in range(2)]

            # ---- init ----
            for hh in range(2):
                for gg in range(2):
                    nc.vector.memset(pkq_g[hh][gg][:], 0.0)
                    nc.vector.memset(hb1_g[hh][gg][:], 0.0)
                nc.vector.memset(c_h[hh][:], 0.0)

            def phase1(wm, h):
                """wm = wave index mod 8."""
                pb = wm % 2
                ps = wm % 4
                p = pkq_g[h][wm // 4][:, wm % 4, :]
                for g in range(4):
                    nc.tensor.matmul(
                        gps_h[h][:, g * HB : (g + 1) * HB],
                        wg_t[:, g, :], p, start=True, stop=True,
                    )
                nc.scalar.activation(sig_p[h][ps][:], gps_h[h][:],
                                     AF.Sigmoid)
                i_s = sig_p[h][ps][:, 0:HB]
                f_s = sig_p[h][ps][:, HB : 2 * HB]
                g_s = sig_p[h][ps][:, 2 * HB : 3 * HB]
                # v = sig_f * c ; u3 = (2*sig_2g - 1) * sig_i = tanh(g)*sig_i
                # Half A's chain runs on DVE, half B's on Pool (no head-of-
                # line blocking between the pipelines) — except half B's v,
                # which is off the wt->u3->c serial path and runs on DVE
                # (idle then), cutting Pool's serial path to tanh(B) by one
                # ~200ns op.
                if h == 0:
                    nc.vector.tensor_scalar(
                        wt_h[h][:], g_s, 2.0, -1.0, OP.mult, OP.add)
                    nc.vector.tensor_tensor(
                        v_h[h][:], f_s, c_h[h][:, 1 - pb, :], OP.mult)
                    nc.vector.tensor_tensor(
                        u3_h[h][:], wt_h[h][:], i_s, OP.mult)
                    nc.vector.tensor_tensor(
                        c_h[h][:, pb, :], u3_h[h][:], v_h[h][:], OP.add)
                else:
                    nc.gpsimd.tensor_scalar(
                        wt_h[h][:], g_s, 2.0, -1.0, OP.mult, OP.add)
                    nc.vector.tensor_tensor(
                        v_h[h][:], f_s, c_h[h][:, 1 - pb, :], OP.mult)
                    nc.gpsimd.tensor_tensor(
                        u3_h[h][:], wt_h[h][:], i_s, OP.mult)
                    nc.gpsimd.tensor_tensor(
                        c_h[h][:, pb, :], u3_h[h][:], v_h[h][:], OP.add)

            def phase2(wm, h):
                pb = wm % 2
                ps = wm % 4
                nc.scalar.activation(tct_p[h][ps][:], c_h[h][:, pb, :],
                                     AF.Tanh)
                w2 = (wm + 1) % 8
                eng = nc.vector if h == 0 else nc.gpsimd
                eng.tensor_tensor(
                    pkq_g[h][w2 // 4][0:96, w2 % 4, :],
                    sig_p[h][ps][0:96, 3 * HB : 4 * HB],
                    tct_p[h][ps][0:96, :], OP.mult,
                )
                eng.tensor_tensor(
                    hb1_g[h][wm // 4][96:128, wm % 4, :],
                    sig_p[h][ps][96:128, 3 * HB : 4 * HB],
                    tct_p[h][ps][96:128, :], OP.mult,
                )

            def xload(grp, s4):
                """load x rows of group grp (waves 4g..4g+3) into slots
                s4..s4+4, one DMA per half (SP for A, ACT queue for B).
                grp: int, or (q_dsexpr, parity) with grp = 2q + parity."""
                for h, eng in ((0, nc.sync), (1, nc.scalar)):
                    if isinstance(grp, int):
                        q0 = grp // 2 + 2
                        src = xd[q0 : q0 + 1, grp % 2, h, :, :, :]
                    else:
                        q, par = grp
                        src = xd[q, par, h, :, :, :]
                    eng.dma_start(pkq_g[h][s4 // 4][96:99, 0:4, :], src)

            def full_wave(wm, wo1i=None, wo2i=None, xl=None, cp=None,
                          dma_q=None):
                """wo1i/wo2i: (ring_tile_idx, col) for out rows t=w-2 / t=w-3.
                cp: (ring_tile_idx, t-col-base-is-implicit) staging copy.
                dma_q: DRAM q index (int or register expr) for an 8-col block.
                """
                for h in range(2):
                    phase1(wm, h)
                # wo2 (closing column t=w-3) MUST precede wo1 (opening
                # column t=w-2): PSUM accumulation groups on a tile must be
                # strictly sequential, never interleaved.
                if wo2i is not None:
                    ii, kk = wo2i
                    s2 = (wm - 1) % 8
                    for h in range(2):
                        nc.tensor.matmul(
                            obr[h][ii][:, kk : kk + 1],
                            hb1_g[h][s2 // 4][:, s2 % 4, :], wo2_t[:],
                            start=False, stop=True, skip_group_check=True)
                if wo1i is not None:
                    ii, kk = wo1i
                    for h in range(2):
                        nc.tensor.matmul(
                            obr[h][ii][:, kk : kk + 1],
                            pkq_g[h][wm // 4][:, wm % 4, :], wo1_t[:],
                            start=True, stop=False, skip_group_check=True)
                for h in range(2):
                    phase2(wm, h)
                if xl is not None:
                    xload(*xl)
                # evac traffic goes AFTER phase2 so it never delays the
                # critical h2 ops in the DVE/Pool queues.
                if cp is not None:
                    ti, sl = cp
                    for h in range(2):
                        nc.vector.tensor_copy(
                            stg[h][:, sl // 2, 4 * (sl % 2) : 4 * (sl % 2) + 4],
                            obr[h][ti][:, 0:4])
                if dma_q is not None:
                    for h, eng in ((0, nc.sync), (1, nc.scalar)):
                        eng.dma_start(
                            outd[h, :, dma_q : dma_q + 2, :]
                            if isinstance(dma_q, int)
                            else outd[h, :, bass.ds(dma_q, 2), :],
                            stg[h][:])

            def wave_kwargs_static(w):
                """out/copy/dma/xload schedule for a statically-indexed wave."""
                kw = {}
                t1 = w - 2
                if 0 <= t1 < s:
                    kw["wo1i"] = ((t1 // 4) % 2, t1 % 4)
                t2 = w - 3
                if 0 <= t2 < s:
                    kw["wo2i"] = ((t2 // 4) % 2, t2 % 4)
                if w % 4 == 3 and (w + 5) // 4 < ngroup:
                    kw["xl"] = ((w + 5) // 4, 0 if w % 8 == 3 else 4)
                if w % 8 == 7 and w >= 7 and w - 7 < s:
                    # ring tile 0 holds t = w-7 .. w-4
                    kw["cp"] = (0, ((w - 7) // 4) % 4)
                if w % 8 == 3 and w >= 11 and w - 7 < s:
                    # ring tile 1 holds t = w-7 .. w-4
                    kw["cp"] = (1, ((w - 7) // 4) % 4)
                    if w % 16 == 3 and w >= 19:
                        kw["dma_q"] = (w - 19) // 8
                return kw

            # ---- prologue: waves 0,1 (skew priming, no out) ----
            xload(0, 0)
            xload(1, 4)
            full_wave(0)
            # zero junk written into hf1/hb0 rows of pkq slot 1 and cf1/cb0
            for hh in range(2):
                nc.vector.memset(pkq_g[hh][0][32:64, 1, :], 0.0)
                nc.vector.memset(pkq_g[hh][0][64:96, 1, :], 0.0)
                nc.vector.memset(c_h[hh][32:64, 0, :], 0.0)
                nc.vector.memset(c_h[hh][64:96, 0, :], 0.0)
            full_wave(1)
            for hh in range(2):
                nc.vector.memset(c_h[hh][96:128, 1, :], 0.0)

            # ---- static waves 2..33 (peel: groups 0..7 compute) ----
            for w in range(2, 34):
                full_wave(w % 8, **wave_kwargs_static(w))

            def dyn_body(iqb, goff):
                """2 groups = 8 waves: G = 2*iqb + goff, w = 4G+2 .. 4G+9.
                iqb is the loop register in 2-group units; every dynamic
                index is a single `iqb + const`."""
                for idx in range(8):
                    w_off = 2 + idx
                    wm = w_off % 8
                    kw = {}
                    kw["wo1i"] = ((idx // 4) % 2, idx % 4)
                    kw["wo2i"] = (((idx - 1) // 4) % 2, (idx - 1) % 4)
                    if wm == 3:
                        kw["xl"] = ((bass.ds(iqb + goff // 2 + 3, 1), 0), 0)
                        kw["cp"] = (1, (goff - 1) % 4)
                        if goff % 4 == 0:
                            kw["dma_q"] = iqb + goff // 2 - 2
                    elif wm == 7:
                        kw["xl"] = ((bass.ds(iqb + goff // 2 + 3, 1), 1), 4)
                        kw["cp"] = (0, goff % 4)
                    full_wave(wm, **kw)

            # ---- main loop: groups 8 .. s//4-2, 8 groups per iteration.
            # The loop register iq counts 2-group units; parity-split xg and
            # 16-column out blocks keep the per-queue dynamic-DMA expression
            # variety at 6 (the AP lowering chokes on too many base+const
            # variants per queue, and walrus cannot codegen gpsimd SWDGE
            # DMAs at all), while 32-wave bodies amortize the per-iteration
            # all-engine barrier.
            with tc.For_i(4, s // 8, 4) as iq:
                for g4 in range(4):
                    dyn_body(iq, 2 * g4)

            # ---- epilogue ----
            # last out row t = s-1: wo2 against hb1 slot (s+1)%8 = 1
            for h in range(2):
                nc.tensor.matmul(
                    obr[h][1][:, 3:4], hb1_g[h][0][:, 1, :], wo2_t[:],
                    start=False, stop=True, skip_group_check=True)
            # evacuate the final 16 columns (t = s-16 .. s-1): staging slots
            # 0..2 were filled in-loop; copy ring tile 1 into slot 3, then
            # one DMA for q8 pair s//8-2.
            for h in range(2):
                nc.vector.tensor_copy(stg[h][:, 1, 4:8], obr[h][1][:, 0:4])
            for h, eng in ((0, nc.sync), (1, nc.scalar)):
                eng.dma_start(
                    outd[h, :, s // 8 - 2 : s // 8, :], stg[h][:])

    if split_waits:
        _split_excess_waits(nc)
    return nc


_NC_CACHE = {}


def _get_nc(s=S, dbg=False):
    key = (s, dbg)
    if key not in _NC_CACHE:
        _NC_CACHE[key] = build_nc(s, dbg)
    return _NC_CACHE[key]


def core_inputs(x, weights, s=S, core=0):
    """Input map for one core. x: [s, B] fp32 (already squeezed)."""
    Wg, wout1, wout2 = build_weights(**weights)
    xs = np.ascontiguousarray(x[:, core * BL : (core + 1) * BL])
    return {"xg": build_xg(xs, s),
            "Wg": Wg, "wout1": wout1, "wout2": wout2}


def run(x, weights, s=S, dbg=False, trace=False):
    """x: [s, B] fp32 (already squeezed); weights: dict of reference arrays."""
    Wg, wout1, wout2 = build_weights(**weights)
    nc = _get_nc(s, dbg)
    in_maps = []
    for c in range(NCORES):
        xs = np.ascontiguousarray(x[:, c * BL : (c + 1) * BL])
        in_maps.append(
            {"xg": build_xg(xs, s),
             "Wg": Wg, "wout1": wout1, "wout2": wout2}
        )
    res = run_bass_kernel_spmd(nc, in_maps, list(range(NCORES)), trace=trace)
    out = np.concatenate(
        [res.results[c]["out"].reshape(BL, s) for c in range(NCORES)], axis=0)
    return out, res


def kernel(x, Wih_f0, Whh_f0, b_f0, Wih_f1, Whh_f1, b_f1,
           Wih_b0, Whh_b0, b_b0, Wih_b1, Whh_b1, b_b1, Wlin, blin, future):
    assert int(future) == 0, "kernel hardcodes future=0"
    x = np.asarray(x, np.float32)
    s, b, _ = x.shape
    assert (s, b) == (S, B)
    weights = dict(
        Wih_f0=np.asarray(Wih_f0, np.float32), Whh_f0=np.asarray(Whh_f0, np.float32),
        b_f0=np.asarray(b_f0, np.float32),
        Wih_f1=np.asarray(Wih_f1, np.float32), Whh_f1=np.asarray(Whh_f1, np.float32),
        b_f1=np.asarray(b_f1, np.float32),
        Wih_b0=np.asarray(Wih_b0, np.float32), Whh_b0=np.asarray(Whh_b0, np.float32),
        b_b0=np.asarray(b_b0, np.float32),
        Wih_b1=np.asarray(Wih_b1, np.float32), Whh_b1=np.asarray(Whh_b1, np.float32),
        b_b1=np.asarray(b_b1, np.float32),
        Wlin=np.asarray(Wlin, np.float32), blin=np.asarray(blin, np.float32),
    )
    out, _ = run(x[:, :, 0], weights, s=S)
    return out
